# revision 33
# baseline (speedup 1.0000x reference)
r"""Boson-sampling probability |Perm(A)|^2 via Glynn's formula on 8 Trainium2 cores.

Math
----
perm(A) = 2^(1-n) * sum_{d in {-1,+1}^n} (prod_i d_i) * prod_j (sum_i d_i A_ij), n=20.
Terms for d and -d are equal, so enumerate d_19 = -1 only and double.

Sign-bit allocation for the remaining 19 bits:
  bits 0..8   -> free axis f (512)       [same on every core]
  bits 9..15  -> partition axis p (128)  [same on every core]
  bits 16..18 -> core c (8)

Row vector V_j(p,f,c) = Cp_c[p,j] + Cf[f,j] with
  Cp_c[p,j] = sum_{i=9..15} d_i(p) A[i,j] + sum_{i=16..18} d_i(c) A[i,j] - A[19,j]
  Cf[f,j]   = sum_{i=0..8} d_i(f) A[i,j]

Split the j-product into groups GA=0..6, GB=7..13, GC=14..19. Each group
product expands over subsets T of the group:
  PG[p,f] = sum_T (prod_{j in T} Cp[p,j]) * (prod_{j in G\T} Cf[f,j])
a bilinear form of rank 2^|G| -> computed on TensorE as fp32 matmuls with
PSUM accumulation (contraction over 2*2^|G| re/im-expanded rows). The
per-term parity prod_i d_i is folded into group A's host tables. VectorE
then combines P = PA*PB*PC (complex) and reduces over f with fused
tensor_tensor_reduce ops; the (128,2) per-core partials are summed on host
in float64.
"""

import numpy as np

N = 20
N_CORES = 8
F = 512           # free size (bits 0..8)
P = 128           # partitions (bits 9..15)
GA = list(range(0, 7))
GB = list(range(7, 14))
GC = list(range(14, 20))

_PROGRAM_CACHE = {}


def _signs(count, nbits):
    v = np.arange(count, dtype=np.int64)[:, None]
    return (((v >> np.arange(nbits)) & 1) * 2.0 - 1.0)  # (count, nbits) float64


def _subset_prods(C):
    """C: (nvals, g) complex128 -> (2^g, nvals); row T = prod_{k: bit k of T} C[:, k]."""
    out = np.ones((1, C.shape[0]), np.complex128)
    for k in range(C.shape[1]):
        out = np.concatenate([out, out * C[None, :, k]], axis=0)
    return out


def _pack_group(U, V):
    """Interleave re/im rows for the paired-contraction matmul layout.

    One shared V table streams through two matmuls; the re/im arithmetic is
    carried by two lhsT variants (contraction rows m = 2T + c):
      vtab[2T]   = Re V[T],  vtab[2T+1]   = Im V[T]
      lhs_re[2T] = Re U[T],  lhs_re[2T+1] = -Im U[T]   (-> PG_re)
      lhs_im[2T] = Im U[T],  lhs_im[2T+1] =  Re U[T]   (-> PG_im)
    """
    nT = U.shape[0]
    lre = np.empty((2 * nT, U.shape[1]), np.float32)
    lre[0::2] = U.real
    lre[1::2] = -U.imag
    lim = np.empty((2 * nT, U.shape[1]), np.float32)
    lim[0::2] = U.imag
    lim[1::2] = U.real
    vtab = np.empty((2 * nT, V.shape[1]), np.float32)
    vtab[0::2] = V.real
    vtab[1::2] = V.imag
    return lre, lim, vtab


def _build_core_tables(A, core):
    """Host tables for one core. A: (20,20) complex128."""
    f_signs = _signs(F, 9)
    p_signs = _signs(P, 7)
    c_signs = _signs(N_CORES, 3)
    par_f = np.prod(f_signs, axis=1)
    par_p = np.prod(p_signs, axis=1)
    par_c = np.prod(c_signs[core])

    Cf = f_signs @ A[0:9, :]                                         # (512, 20)
    Cp = p_signs @ A[9:16, :] + (c_signs[core] @ A[16:19, :] - A[19, :])[None, :]

    out = {}
    for name, G in (("A", GA), ("B", GB), ("C", GC)):
        U = _subset_prods(Cp[:, G])          # (2^g, 128)
        VV = _subset_prods(Cf[:, G])         # (2^g, 512)
        V = VV[::-1]                         # complement subset: T -> 2^g-1-T
        if name == "A":
            # fold full parity: par_p(p) * par_f(f) * par_c * (-1 for d19)
            U = U * (par_p[None, :] * (-par_c))
            V = V * par_f[None, :]
        lre, lim, vtab = _pack_group(U, V)
        nchunks = lre.shape[0] // 128
        packed = np.concatenate([lre, lim, vtab], axis=1)  # (2^g*2, 128+128+512)
        out["tab" + name] = np.ascontiguousarray(
            packed.reshape(nchunks, 128, 2 * P + F).astype(np.float16))
    return out


def _build_program():
    if "prog" in _PROGRAM_CACHE:
        return _PROGRAM_CACHE["prog"]

    from contextlib import ExitStack
    from concourse import bass, mybir

    f32 = mybir.dt.float32
    # FP16 tables: native 1-cycle/row PE path and half the DMA bytes of
    # fp32. Only the table values are rounded (2^-11); products accumulate
    # exactly in fp32 PSUM -> measured ~1e-4 end-to-end error, better than
    # float32r (~3e-3). Flip to mybir.dt.float32 for the exact fallback.
    mm_dt = mybir.dt.float16
    mul = mybir.AluOpType.mult
    add = mybir.AluOpType.add
    nc = bass.Bass()

    # DRAM parameters (per-core data is supplied via in_maps; same program on
    # all cores). Each group is one packed tensor [lhsT_re | lhsT_im | V].
    W = 2 * P + F
    groups = (("A", 2), ("B", 2), ("C", 1))
    dram = {}
    for g, nch in groups:
        dram[g] = nc.declare_dram_parameter("tab" + g, [nch, 128, W], mm_dt, isOutput=False)
    out_dram = nc.declare_dram_parameter("out", [P, 4], f32, isOutput=True)

    es = ExitStack()
    with es:
        block = es.enter_context(nc.Block(no_gpsimd_drain=True))
        # one semaphore per load DMA: cross-SDMA-engine completion order is
        # not guaranteed, so thresholds on a shared semaphore would race.
        dsem = [es.enter_context(nc.semaphore(f"dma{i}")) for i in range(5)]
        pe_sem = es.enter_context(nc.semaphore("pe_sem"))
        act_sem = es.enter_context(nc.semaphore("act_sem"))
        dve_sem = es.enter_context(nc.semaphore("dve_sem"))
        gp_sem = es.enter_context(nc.semaphore("gp_sem"))
        warm_sem = es.enter_context(nc.semaphore("warm_sem"))

        sb = {}
        for g, nch in groups:
            sb[g] = es.enter_context(nc.sbuf_tensor("sb_tab" + g, [128, nch, W], mm_dt))
        f16 = mybir.dt.float16
        names = ["sPAre", "sPAim", "sPBre", "sPBim", "sPCre", "sPCim",
                 "t1", "t2", "t3", "t4", "U_", "W_",
                 "scr1", "scr2", "scr3", "scr4"]
        wt = {n: es.enter_context(nc.sbuf_tensor(n, [P, F], f16)) for n in names}
        racc = [es.enter_context(nc.sbuf_tensor(f"racc{i}", [P, 1], f32)) for i in range(4)]
        out_t = es.enter_context(nc.sbuf_tensor("out_t", [P, 4], f32))
        dummy = es.enter_context(nc.sbuf_tensor("actwarm", [P, 2], f32))
        pg = {}
        for g in ("A", "B", "C"):
            for comp in ("re", "im"):
                pg[g + comp] = es.enter_context(
                    nc.psum_tensor("pg" + g + comp, [P, F], f32))

        # DMA order = consumption order: C gates the ACT evictions (first),
        # B gates the DVE M-stage, A gates only the final fused accumulates
        # (lands last). All five loads go FIFO down the SP HWDGE ring.
        chunk_list = [("C", 0), ("B", 0), ("B", 1), ("A", 0), ("A", 1)]

        @block.sync
        def _(sync):
            # B and A chunks FIFO down the SP ring, in consumption order
            for i, (g, k) in enumerate(chunk_list[1:]):
                sync.dma_start(sb[g][:, k, :], dram[g][k]).then_inc(dsem[i + 1], 16)

        @block.scalar
        def _(act):
            # C immediately on the ACT ring — small, lands first, and only
            # briefly shares HBM with B0
            act.dma_start(sb["C"][:, 0, :], dram["C"][0]).then_inc(dsem[0], 16)
            # touch ACT once before any gating wait so walrus's activation
            # table load happens during the DMA head, off the critical path
            act.wait_ge(warm_sem, 1)
            act.copy(dummy[:, 1:2], dummy[:, 0:1])
            # PSUM->SBUF evictions, casting fp32 -> fp16 so every DVE combine
            # op runs in the 2x packed perf mode on SBUF operands
            act.wait_ge(pe_sem, 2)
            act.copy(wt["sPCre"][:, :], pg["Cre"][:, :]).then_inc(act_sem, 1)
            act.copy(wt["sPCim"][:, :], pg["Cim"][:, :]).then_inc(act_sem, 1)
            act.wait_ge(pe_sem, 5)
            act.copy(wt["sPBre"][:, :], pg["Bre"][:, :]).then_inc(act_sem, 1)
            act.wait_ge(pe_sem, 6)
            act.copy(wt["sPBim"][:, :], pg["Bim"][:, :]).then_inc(act_sem, 1)
            # store; the end-of-block engine drains cover DMA completion
            act.wait_ge(dve_sem, 10)
            act.dma_start(out_dram[:], out_t[:, :]).then_inc(dsem[0], 16)

        @block.tensor
        def _(pe):
            # matmul order: Cre Cim | Bre0 Bim0 Bre1 Bim1 | Are0 Aim0 Are1 Aim1
            # pe_sem: PC done at 2, PBre at 5, PBim at 6, PAre at 9, PAim at 10
            for i, (g, k) in enumerate(chunk_list):
                pe.wait_ge(dsem[i], 16)
                nch = 2 if g in ("A", "B") else 1
                for comp, lo in (("re", 0), ("im", P)):
                    pe.matmul(
                        pg[g + comp][:, :],
                        sb[g][:, k, lo:lo + P],
                        sb[g][:, k, 2 * P:2 * P + F],
                        start=(k == 0),
                        stop=(k == nch - 1),
                    ).then_inc(pe_sem, 1)

        @block.vector
        def _(v):
            # M = PC*PB, then fused dot-products against PA. All operands are
            # fp16 SBUF -> DVE 2x packed mode. Standalone self-waits make
            # same-engine RAW/WAW explicit.
            v.memset(dummy[:, 0:1], 0.0).then_inc(warm_sem, 1)
            v.wait_ge(act_sem, 3)
            v.tensor_mul(wt["t1"][:, :], wt["sPCre"][:, :], wt["sPBre"][:, :]).then_inc(dve_sem, 1)
            v.tensor_mul(wt["t4"][:, :], wt["sPCim"][:, :], wt["sPBre"][:, :]).then_inc(dve_sem, 1)
            v.wait_ge(act_sem, 4)
            v.tensor_mul(wt["t2"][:, :], wt["sPCim"][:, :], wt["sPBim"][:, :]).then_inc(dve_sem, 1)
            v.tensor_mul(wt["t3"][:, :], wt["sPCre"][:, :], wt["sPBim"][:, :]).then_inc(dve_sem, 1)
            v.wait_ge(dve_sem, 3)
            v.tensor_sub(wt["U_"][:, :], wt["t1"][:, :], wt["t2"][:, :]).then_inc(dve_sem, 1)
            v.wait_ge(dve_sem, 4)
            v.tensor_add(wt["W_"][:, :], wt["t3"][:, :], wt["t4"][:, :]).then_inc(dve_sem, 1)
            # out cols: 0 = sum U*PAre, 1 = sum W*PAim, 2 = sum U*PAim,
            # 3 = sum W*PAre ; host computes re = c0-c1, im = c2+c3.
            # STT has no 16-bit fast uop (1x either way), so read PA from
            # PSUM directly — no eviction needed.
            v.wait_ge(pe_sem, 9)
            v.wait_ge(dve_sem, 5)
            v.scalar_tensor_tensor(
                wt["scr1"][:, :], wt["U_"][:, :], 1.0, pg["Are"][:, :],
                mul, mul, accum_out=out_t[:, 0:1]).then_inc(dve_sem, 1)
            v.wait_ge(pe_sem, 10)
            v.wait_ge(dve_sem, 6)
            v.scalar_tensor_tensor(
                wt["scr2"][:, :], wt["W_"][:, :], 1.0, pg["Aim"][:, :],
                mul, mul, accum_out=out_t[:, 1:2]).then_inc(dve_sem, 1)
            v.scalar_tensor_tensor(
                wt["scr3"][:, :], wt["U_"][:, :], 1.0, pg["Aim"][:, :],
                mul, mul, accum_out=out_t[:, 2:3]).then_inc(dve_sem, 1)
            v.scalar_tensor_tensor(
                wt["scr4"][:, :], wt["W_"][:, :], 1.0, pg["Are"][:, :],
                mul, mul, accum_out=out_t[:, 3:4]).then_inc(dve_sem, 1)

    nc.finalize()
    _PROGRAM_CACHE["prog"] = nc
    return nc


def kernel(A_real, A_imag, _collect=None):
    from concourse.bass_utils import run_bass_kernel_spmd

    A = np.asarray(A_real, np.float64) + 1j * np.asarray(A_imag, np.float64)
    nc = _build_program()
    in_maps = [_build_core_tables(A, c) for c in range(N_CORES)]

    kwargs = dict(_collect or {})
    res = run_bass_kernel_spmd(nc, in_maps, core_ids=list(range(N_CORES)), **kwargs)
    if _collect is not None:
        _collect["results"] = res

    total = np.complex128(0)
    for r in res.results:
        o = np.asarray(r["out"], np.float64)
        total += (o[:, 0] - o[:, 1]).sum() + 1j * (o[:, 2] + o[:, 3]).sum()

    perm = total * 2.0 * (2.0 ** (1 - N))
    ans = (perm.conjugate() * perm).real
    return np.asarray(ans, np.float32)


# revision 34
# speedup vs baseline: 1.0543x; 1.0543x over previous
r"""Boson-sampling probability |Perm(A)|^2 via Glynn's formula on 8 Trainium2 cores.

Math
----
perm(A) = 2^(1-n) * sum_{d in {-1,+1}^n} (prod_i d_i) * prod_j (sum_i d_i A_ij), n=20.
Terms for d and -d are equal, so enumerate d_19 = -1 only and double.

Sign-bit allocation for the remaining 19 bits:
  bits 0..8   -> free axis f (512)       [same on every core]
  bits 9..15  -> partition axis p (128)  [same on every core]
  bits 16..18 -> core c (8)

Row vector V_j(p,f,c) = Cp_c[p,j] + Cf[f,j] with
  Cp_c[p,j] = sum_{i=9..15} d_i(p) A[i,j] + sum_{i=16..18} d_i(c) A[i,j] - A[19,j]
  Cf[f,j]   = sum_{i=0..8} d_i(f) A[i,j]

Split the j-product into groups GA=0..6, GB=7..13, GC=14..19. Each group
product expands over subsets T of the group:
  PG[p,f] = sum_T (prod_{j in T} Cp[p,j]) * (prod_{j in G\T} Cf[f,j])
a bilinear form of rank 2^|G| -> computed on TensorE as fp32 matmuls with
PSUM accumulation (contraction over 2*2^|G| re/im-expanded rows). The
per-term parity prod_i d_i is folded into group A's host tables. VectorE
then combines P = PA*PB*PC (complex) and reduces over f with fused
tensor_tensor_reduce ops; the (128,2) per-core partials are summed on host
in float64.
"""

import numpy as np

N = 20
N_CORES = 8
F = 512           # free size (bits 0..8)
P = 128           # partitions (bits 9..15)
GA = list(range(0, 7))
GB = list(range(7, 14))
GC = list(range(14, 20))

_PROGRAM_CACHE = {}


def _signs(count, nbits):
    v = np.arange(count, dtype=np.int64)[:, None]
    return (((v >> np.arange(nbits)) & 1) * 2.0 - 1.0)  # (count, nbits) float64


def _subset_prods(C):
    """C: (nvals, g) complex128 -> (2^g, nvals); row T = prod_{k: bit k of T} C[:, k]."""
    out = np.ones((1, C.shape[0]), np.complex128)
    for k in range(C.shape[1]):
        out = np.concatenate([out, out * C[None, :, k]], axis=0)
    return out


def _pack_group(U, V):
    """Interleave re/im rows for the paired-contraction matmul layout.

    One shared V table streams through two matmuls; the re/im arithmetic is
    carried by two lhsT variants (contraction rows m = 2T + c):
      vtab[2T]   = Re V[T],  vtab[2T+1]   = Im V[T]
      lhs_re[2T] = Re U[T],  lhs_re[2T+1] = -Im U[T]   (-> PG_re)
      lhs_im[2T] = Im U[T],  lhs_im[2T+1] =  Re U[T]   (-> PG_im)
    """
    nT = U.shape[0]
    lre = np.empty((2 * nT, U.shape[1]), np.float32)
    lre[0::2] = U.real
    lre[1::2] = -U.imag
    lim = np.empty((2 * nT, U.shape[1]), np.float32)
    lim[0::2] = U.imag
    lim[1::2] = U.real
    vtab = np.empty((2 * nT, V.shape[1]), np.float32)
    vtab[0::2] = V.real
    vtab[1::2] = V.imag
    return lre, lim, vtab


def _build_core_tables(A, core):
    """Host tables for one core. A: (20,20) complex128."""
    f_signs = _signs(F, 9)
    p_signs = _signs(P, 7)
    c_signs = _signs(N_CORES, 3)
    par_f = np.prod(f_signs, axis=1)
    par_p = np.prod(p_signs, axis=1)
    par_c = np.prod(c_signs[core])

    Cf = f_signs @ A[0:9, :]                                         # (512, 20)
    Cp = p_signs @ A[9:16, :] + (c_signs[core] @ A[16:19, :] - A[19, :])[None, :]

    out = {}
    for name, G in (("A", GA), ("B", GB), ("C", GC)):
        U = _subset_prods(Cp[:, G])          # (2^g, 128)
        VV = _subset_prods(Cf[:, G])         # (2^g, 512)
        V = VV[::-1]                         # complement subset: T -> 2^g-1-T
        if name == "A":
            # fold full parity: par_p(p) * par_f(f) * par_c * (-1 for d19)
            U = U * (par_p[None, :] * (-par_c))
            V = V * par_f[None, :]
        lre, lim, vtab = _pack_group(U, V)
        nchunks = lre.shape[0] // 128
        packed = np.concatenate([lre, lim, vtab], axis=1)  # (2^g*2, 128+128+512)
        out["tab" + name] = np.ascontiguousarray(
            packed.reshape(nchunks, 128, 2 * P + F).astype(np.float16))
    return out


def _build_program():
    if "prog" in _PROGRAM_CACHE:
        return _PROGRAM_CACHE["prog"]

    from contextlib import ExitStack
    from concourse import bass, mybir

    f32 = mybir.dt.float32
    # FP16 tables: native 1-cycle/row PE path and half the DMA bytes of
    # fp32. Only the table values are rounded (2^-11); products accumulate
    # exactly in fp32 PSUM -> measured ~1e-4 end-to-end error, better than
    # float32r (~3e-3). Flip to mybir.dt.float32 for the exact fallback.
    mm_dt = mybir.dt.float16
    mul = mybir.AluOpType.mult
    add = mybir.AluOpType.add
    nc = bass.Bass()

    # DRAM parameters (per-core data is supplied via in_maps; same program on
    # all cores). Each group is one packed tensor [lhsT_re | lhsT_im | V].
    W = 2 * P + F
    groups = (("A", 2), ("B", 2), ("C", 1))
    dram = {}
    for g, nch in groups:
        dram[g] = nc.declare_dram_parameter("tab" + g, [nch, 128, W], mm_dt, isOutput=False)
    out_dram = nc.declare_dram_parameter("out", [P, 4], f32, isOutput=True)

    es = ExitStack()
    with es:
        block = es.enter_context(nc.Block(no_gpsimd_drain=True))
        # one semaphore per load DMA: cross-SDMA-engine completion order is
        # not guaranteed, so thresholds on a shared semaphore would race.
        dsem = [es.enter_context(nc.semaphore(f"dma{i}")) for i in range(5)]
        pe_sem = es.enter_context(nc.semaphore("pe_sem"))
        act_sem = es.enter_context(nc.semaphore("act_sem"))
        dve_sem = es.enter_context(nc.semaphore("dve_sem"))
        gp_sem = es.enter_context(nc.semaphore("gp_sem"))
        warm_sem = es.enter_context(nc.semaphore("warm_sem"))

        sb = {}
        for g, nch in groups:
            sb[g] = es.enter_context(nc.sbuf_tensor("sb_tab" + g, [128, nch, W], mm_dt))
        f16 = mybir.dt.float16
        names = ["sPAre", "sPAim", "sPBre", "sPBim", "sPCre", "sPCim",
                 "t1", "t2", "t3", "t4", "U_", "W_",
                 "scr1", "scr2", "scr3", "scr4"]
        wt = {n: es.enter_context(nc.sbuf_tensor(n, [P, F], f16)) for n in names}
        racc = [es.enter_context(nc.sbuf_tensor(f"racc{i}", [P, 1], f32)) for i in range(4)]
        out_t = es.enter_context(nc.sbuf_tensor("out_t", [P, 4], f32))
        dummy = es.enter_context(nc.sbuf_tensor("actwarm", [P, 2], f32))
        pg = {}
        for g in ("A", "B", "C"):
            for comp in ("re", "im"):
                pg[g + comp] = es.enter_context(
                    nc.psum_tensor("pg" + g + comp, [P, F], f32))

        # DMA order = consumption order: C gates the ACT evictions (first),
        # B gates the DVE M-stage, A gates only the final fused accumulates
        # (lands last). All five loads go FIFO down the SP HWDGE ring.
        chunk_list = [("C", 0), ("B", 0), ("B", 1), ("A", 0), ("A", 1)]

        @block.sync
        def _(sync):
            # B and A chunks FIFO down the SP ring, in consumption order
            for i, (g, k) in enumerate(chunk_list[1:]):
                sync.dma_start(sb[g][:, k, :], dram[g][k]).then_inc(dsem[i + 1], 16)

        @block.scalar
        def _(act):
            # C immediately on the ACT ring — small, lands first, and only
            # briefly shares HBM with B0
            act.dma_start(sb["C"][:, 0, :], dram["C"][0]).then_inc(dsem[0], 16)
            # touch ACT once before any gating wait so walrus's activation
            # table load happens during the DMA head, off the critical path
            act.wait_ge(warm_sem, 1)
            act.copy(dummy[:, 1:2], dummy[:, 0:1])
            # PSUM->SBUF fp16 evictions of PB (PC is evicted by the DVE,
            # which is otherwise idle earlier)
            act.wait_ge(pe_sem, 4)
            act.copy(wt["sPBre"][:, :], pg["Bre"][:, :]).then_inc(act_sem, 1)
            act.wait_ge(pe_sem, 6)
            act.copy(wt["sPBim"][:, :], pg["Bim"][:, :]).then_inc(act_sem, 1)
            # store; the end-of-block engine drains cover DMA completion
            act.wait_ge(dve_sem, 12)
            act.dma_start(out_dram[:], out_t[:, :]).then_inc(dsem[0], 16)

        @block.tensor
        def _(pe):
            # per-bank contiguous matmul order so each accumulation group
            # closes as early as possible:
            #   Cre Cim | Bre0 Bre1 Bim0 Bim1 | Are0 Are1 Aim0 Aim1
            # pe_sem: PC done at 2, PBre at 4, PBim at 6, PAre at 8, PAim at 10
            def mm(g, comp, k, nch, w):
                if w is not None:
                    pe.wait_ge(dsem[w], 16)
                lo = 0 if comp == "re" else P
                pe.matmul(
                    pg[g + comp][:, :],
                    sb[g][:, k, lo:lo + P],
                    sb[g][:, k, 2 * P:2 * P + F],
                    start=(k == 0),
                    stop=(k == nch - 1),
                ).then_inc(pe_sem, 1)
            mm("C", "re", 0, 1, 0)
            mm("C", "im", 0, 1, None)
            mm("B", "re", 0, 2, 1)
            mm("B", "re", 1, 2, 2)
            mm("B", "im", 0, 2, None)
            mm("B", "im", 1, 2, None)
            mm("A", "re", 0, 2, 3)
            mm("A", "re", 1, 2, 4)
            mm("A", "im", 0, 2, None)
            mm("A", "im", 1, 2, None)

        @block.vector
        def _(v):
            # M = PC*PB in fp16 2x mode; fused 1x dot-products against PA in
            # PSUM. Standalone self-waits make same-engine RAW/WAW explicit.
            v.memset(dummy[:, 0:1], 0.0).then_inc(warm_sem, 1)
            # self-evict PC to fp16 SBUF (2x-mode copies) while B still loads
            v.wait_ge(pe_sem, 2)
            v.tensor_copy(wt["sPCre"][:, :], pg["Cre"][:, :]).then_inc(dve_sem, 1)
            v.tensor_copy(wt["sPCim"][:, :], pg["Cim"][:, :]).then_inc(dve_sem, 1)
            v.wait_ge(act_sem, 1)
            v.wait_ge(dve_sem, 1)
            v.tensor_mul(wt["t1"][:, :], wt["sPCre"][:, :], wt["sPBre"][:, :]).then_inc(dve_sem, 1)
            v.wait_ge(dve_sem, 2)
            v.tensor_mul(wt["t4"][:, :], wt["sPCim"][:, :], wt["sPBre"][:, :]).then_inc(dve_sem, 1)
            v.wait_ge(act_sem, 2)
            v.tensor_mul(wt["t2"][:, :], wt["sPCim"][:, :], wt["sPBim"][:, :]).then_inc(dve_sem, 1)
            v.tensor_mul(wt["t3"][:, :], wt["sPCre"][:, :], wt["sPBim"][:, :]).then_inc(dve_sem, 1)
            v.wait_ge(dve_sem, 5)
            v.tensor_sub(wt["U_"][:, :], wt["t1"][:, :], wt["t2"][:, :]).then_inc(dve_sem, 1)
            v.wait_ge(dve_sem, 6)
            v.tensor_add(wt["W_"][:, :], wt["t3"][:, :], wt["t4"][:, :]).then_inc(dve_sem, 1)
            # out cols: 0 = sum U*PAre, 1 = sum W*PAim, 2 = sum U*PAim,
            # 3 = sum W*PAre ; host computes re = c0-c1, im = c2+c3.
            # STT has no 16-bit fast uop (1x either way), so read PA from
            # PSUM directly — no eviction needed. U-consumers first (U is
            # ready one op before W).
            v.wait_ge(pe_sem, 8)
            v.wait_ge(dve_sem, 7)
            v.scalar_tensor_tensor(
                wt["scr1"][:, :], wt["U_"][:, :], 1.0, pg["Are"][:, :],
                mul, mul, accum_out=out_t[:, 0:1]).then_inc(dve_sem, 1)
            v.wait_ge(pe_sem, 10)
            v.wait_ge(dve_sem, 8)
            v.scalar_tensor_tensor(
                wt["scr3"][:, :], wt["U_"][:, :], 1.0, pg["Aim"][:, :],
                mul, mul, accum_out=out_t[:, 2:3]).then_inc(dve_sem, 1)
            v.scalar_tensor_tensor(
                wt["scr2"][:, :], wt["W_"][:, :], 1.0, pg["Aim"][:, :],
                mul, mul, accum_out=out_t[:, 1:2]).then_inc(dve_sem, 1)
            v.scalar_tensor_tensor(
                wt["scr4"][:, :], wt["W_"][:, :], 1.0, pg["Are"][:, :],
                mul, mul, accum_out=out_t[:, 3:4]).then_inc(dve_sem, 1)

    nc.finalize()
    _PROGRAM_CACHE["prog"] = nc
    return nc


def kernel(A_real, A_imag, _collect=None):
    from concourse.bass_utils import run_bass_kernel_spmd

    A = np.asarray(A_real, np.float64) + 1j * np.asarray(A_imag, np.float64)
    nc = _build_program()
    in_maps = [_build_core_tables(A, c) for c in range(N_CORES)]

    kwargs = dict(_collect or {})
    res = run_bass_kernel_spmd(nc, in_maps, core_ids=list(range(N_CORES)), **kwargs)
    if _collect is not None:
        _collect["results"] = res

    total = np.complex128(0)
    for r in res.results:
        o = np.asarray(r["out"], np.float64)
        total += (o[:, 0] - o[:, 1]).sum() + 1j * (o[:, 2] + o[:, 3]).sum()

    perm = total * 2.0 * (2.0 ** (1 - N))
    ans = (perm.conjugate() * perm).real
    return np.asarray(ans, np.float32)
